# revision 81
# baseline (speedup 1.0000x reference)
"""Causal dense attention (key=value) on 8 TRN2 NeuronCores.

Reference semantics (B=4, T=2048, D=1024, fp32):
    scores  = Q @ V^T                      [B, T, T]
    scores -= 1e9 * (~tril)                causal mask
    W       = softmax(scores, axis=-1)
    out     = W @ V                        [B, T, D]

Sharding: 2 cores per batch; each batch's 16 causal q-tiles (128 rows)
split odd/even across the pair so all 8 cores run one SPMD program with
the padded kv-extent schedule EXT = [256, 512, ..., 2048].

v2 (cost-model driven; 100942 -> 75047 ns):
  - fp16 operands everywhere (mm1, mm2, transposes, output); host converts.
  - inputs staged as partition-major SBUF images packed into 16 wave
    tensors, so every DMA is one fully-contiguous transfer, sequenced so
    each slot's q-tile lands just before the V^T columns it needs.
  - all input DMAs issued up-front on the SP queue; output DMAs queue
    behind them.
  - PE warm-up transposes cover the initial DMA latency so real matmuls
    start at full clock (cost model ramps 0.65 -> 1.2 -> 2.4 GHz over
    3us of continuous PE busy).
  - additive causal mask (-30000, fp16-safe) applied by DVE into PSUM;
    one [128, 256] mask tile serves all slots (threshold is
    slot-independent per core group).
  - per-slot pipeline (LAG=3): slot j's W^T transpose groups are emitted
    at the pass boundaries of slot j-1's mm2, one group per boundary, so
    each group's PSUM->SBUF copy hides behind ready mm2 work instead of
    clogging the PE wait queue.
  - drain ends on the largest slot (its mm2 hides all earlier output
    DMAs) with a narrowed 128-column final pass to minimize the post-PE
    evac -> DMA -> semaphore tail.
"""

import numpy as np

import concourse.bass as bass
import concourse.mybir as mybir
from concourse import bacc, tile
from concourse.bass_utils import run_bass_kernel_spmd
from concourse.masks import make_identity

B, T, D = 4, 2048, 1024
NCORES = 8
NSLOT = 8
EXT = [256 * (j + 1) for j in range(NSLOT)]  # kv extent per slot
# ascending: tiny slots first (fast, low-bandwidth ramp-in), biggest last
ORDER = [0, 1, 2, 3, 4, 5, 6, 7]
LAG = 3
NWARM = 34
MASK_VAL = -30000.0

F32 = mybir.dt.float32
F16 = mybir.dt.float16

# Input DMA waves: each is ONE contiguous partition-major DMA packing
# several logical chunks. Few, larger waves keep the SP sequencer's DMA
# issue phase short so the W^T transpose-DMAs never interleave ahead of
# inputs on the shared HWDGE completion rings.
#   chunk kinds: ("mask",), ("qt", j), ("vt", c0, w), ("vn", r0, nrow)
WAVES = [
    [("qt", 0), ("vt", 0, 128), ("vt", 128, 128)],
    [("mask",), ("qt", 1)],
    [("vt", 256, 128)],
    [("vt", 384, 128)],
    [("qt", 2)],
    [("vt", 512, 256)],
    [("qt", 3)],
    [("vt", 768, 256)],
    [("qt", 4)],
    [("vt", 1024, 256), ("vn", 0, 2)],
    [("qt", 5), ("vt", 1280, 256), ("vn", 256, 2)],
    [("vn", 512, 4)],
    [("qt", 6), ("vt", 1536, 256)],
    [("qt", 7), ("vt", 1792, 256)],
    [("vn", 1024, 4)],
    [("vn", 1536, 4)],
]


def _chunk_cols(ch):
    if ch[0] == "mask":
        return 256
    if ch[0] == "qt":
        return 8 * 128
    if ch[0] == "vt":
        return 8 * ch[2]
    return ch[2] * D  # vn


def _wave_cols(wave):
    return sum(_chunk_cols(ch) for ch in wave)


def _tiles_for_core(c):
    """q-tile index (within the batch) for each slot, for core c."""
    if c < 4:
        return [2 * j + 1 for j in range(NSLOT)]  # extents exactly EXT
    return [2 * j for j in range(NSLOT)]  # extents EXT - 128 (padded)


VT_SPANS = [(ch[1], ch[2]) for wave in WAVES for ch in wave if ch[0] == "vt"]


def _segments(c0, w):
    """Split window [c0, c0+w) at vt-chunk boundaries -> (span, off, g0, gw)."""
    segs = []
    for si, (s0, sw) in enumerate(VT_SPANS):
        lo = max(c0, s0)
        hi = min(c0 + w, s0 + sw)
        if lo < hi:
            segs.append((si, lo - s0, lo - c0, hi - lo))
    return segs


def _build_program():
    nc = bacc.Bacc("TRN2", target_bir_lowering=False)

    wave_d = [nc.dram_tensor(f"w{k}", [128, _wave_cols(wave)], F16,
                             kind="ExternalInput")
              for k, wave in enumerate(WAVES)]
    o_d = nc.dram_tensor("o", [NSLOT * 128, D], F16, kind="ExternalOutput")

    with tile.TileContext(nc) as tc:
        with (
            tc.tile_pool(name="const", bufs=1) as constp,
            tc.tile_pool(name="vt", bufs=1) as vtp,
            tc.tile_pool(name="w", bufs=4) as wp,
            tc.tile_pool(name="wt", bufs=8) as wtp,
            tc.tile_pool(name="osb", bufs=8) as op,
            tc.tile_pool(name="stats", bufs=24) as statp,
            tc.tile_pool(name="ps_s", bufs=4, space="PSUM") as ps_s,
            tc.tile_pool(name="ps_t", bufs=2, space="PSUM") as ps_t,
            tc.tile_pool(name="ps_o", bufs=2, space="PSUM") as ps_o,
        ):
            # warm-source: zeros are fine, warm-up results are never read
            warmsrc = constp.tile([128, 128], F16, tag="warmsrc")
            nc.gpsimd.memset(warmsrc[:], 0.0)

            ident = constp.tile([128, 128], F16, tag="ident")
            make_identity(nc, ident[:])

            # ACT exp-table warm-up during initial DMAs
            warm_a = statp.tile([128, 1], F32, tag="warma")
            nc.gpsimd.memset(warm_a[:], 0.0)
            nc.scalar.activation(warm_a[:], warm_a[:],
                                 mybir.ActivationFunctionType.Exp)

            # ---- all input DMAs, in slot-consumption order -------------
            qts = {}   # j -> AP [128, 8*128]   (d8-major q-tile image)
            vtc = []   # vt span idx -> (tile, off, w)
            vnb = {}   # 128-row block index -> AP [128, D]
            maskc = None

            for k, wave in enumerate(WAVES):
                t_ = vtp.tile([128, _wave_cols(wave)], F16, tag=f"wv{k}")
                nc.sync.dma_start(t_[:], wave_d[k][:])
                off = 0
                for ch in wave:
                    cols = _chunk_cols(ch)
                    if ch[0] == "mask":
                        maskc = t_[:, off:off + 256]
                    elif ch[0] == "qt":
                        qts[ch[1]] = (t_, off)
                    elif ch[0] == "vt":
                        vtc.append((t_, off, ch[2]))
                    else:  # vn
                        for i in range(ch[2]):
                            vnb[ch[1] // 128 + i] = t_[:, off + i * D:
                                                       off + (i + 1) * D]
                    off += cols

            # ---- PE warm-up: junk transposes cover the DMA head --------
            for _ in range(NWARM):
                t_ps = ps_t.tile([128, 512], F16, tag="tp",
                                 padded_shape=[128, 1024])
                nc.tensor.transpose(t_ps[:, 0:128], warmsrc[:], warmsrc[:])

            def emit_junk(n):
                """junk transposes: keep the PE queue fed across known
                data-bound head gaps (53ns each, results never read)."""
                for _ in range(n):
                    t_ps = ps_t.tile([128, 512], F16, tag="tp",
                                     padded_shape=[128, 1024])
                    nc.tensor.transpose(t_ps[:, 0:128], warmsrc[:],
                                        warmsrc[:])

            def emit_front(j, junk_mid=0):
                """mm1 + softmax stats + exp for slot j."""
                E = EXT[j]
                windows = [(c0, min(512, E - c0)) for c0 in range(0, E, 512)]
                nmax = None
                s_list = []
                for c0, w in windows:
                    s_ = ps_s.tile([128, 512], F32, tag="sw")
                    last = (c0 + w == E)
                    segs = _segments(c0, w)
                    qt_t, qt_off = qts[j]
                    for si, (sp, off, g0, gw) in enumerate(segs):
                        vt_t, vt_off, vw = vtc[sp]
                        for d8 in range(8):
                            nc.tensor.matmul(
                                s_[:, g0:g0 + gw],
                                qt_t[:, qt_off + d8 * 128:
                                     qt_off + (d8 + 1) * 128],
                                vt_t[:, vt_off + d8 * vw + off:
                                     vt_off + d8 * vw + off + gw],
                                start=(d8 == 0 and g0 == 0),
                                stop=(si == len(segs) - 1 and d8 == 7),
                            )
                    # additive causal mask on the last 256 columns (DVE)
                    if last:
                        nc.vector.tensor_add(s_[:, w - 256:w],
                                             s_[:, w - 256:w], maskc[:])
                    nm = statp.tile([128, 1], F32, tag="nm")
                    nc.vector.reduce_max(nm[:], s_[:, :w],
                                         axis=mybir.AxisListType.X, negate=True)
                    if nmax is None:
                        nmax = nm
                    else:
                        nm2 = statp.tile([128, 1], F32, tag="nmc")
                        nc.vector.tensor_tensor(nm2[:], nmax[:], nm[:],
                                                op=mybir.AluOpType.min)
                        nmax = nm2
                    s_list.append((c0, w, s_))

                w_sb = wp.tile([128, E], F16, tag="w")
                rsum = None
                for c0, w, s_ in s_list:
                    rs = statp.tile([128, 1], F32, tag="rs")
                    nc.scalar.activation(
                        w_sb[:, c0:c0 + w], s_[:, :w],
                        mybir.ActivationFunctionType.Exp,
                        bias=nmax[:], accum_out=rs[:],
                    )
                    if rsum is None:
                        rsum = rs
                    else:
                        rs2 = statp.tile([128, 1], F32, tag="rsc")
                        nc.vector.tensor_add(rs2[:], rsum[:], rs[:])
                        rsum = rs2
                return [j, w_sb, rsum, None, None]

            def prep_back(state):
                """1/rowsum (deferred off the hot DVE stats path) and the
                W^T group list for a finished slot."""
                j, w_sb, rsum, _, _ = state
                rinv = statp.tile([128, 1], F32, tag="rinv")
                nc.vector.reciprocal(rinv[:], rsum[:])
                state[3] = rinv
                state[4] = []

            def emit_T_group(state, g0):
                """PE-transpose one group of 4 W blocks + DVE copy to SBUF."""
                j, w_sb, rsum, rinv, wts = state
                nblk = EXT[j] // 128
                gn = min(4, nblk - g0)
                t_ps = ps_t.tile([128, 512], F16, tag="tp",
                                 padded_shape=[128, 1024])
                for bi in range(gn):
                    blk = g0 + bi
                    nc.tensor.transpose(
                        t_ps[:, bi * 128:(bi + 1) * 128],
                        w_sb[:, blk * 128:(blk + 1) * 128],
                        ident[:],
                    )
                wt = wtp.tile([128, 512], F16, tag="wt")
                nc.vector.tensor_copy(wt[:, :gn * 128], t_ps[:, :gn * 128])
                wts.append(wt)

            def emit_back_T(state):
                prep_back(state)
                for g0 in range(0, EXT[state[0]] // 128, 4):
                    emit_T_group(state, g0)

            def emit_back_mm2(state, T_state=None, fine_tail=False):
                """mm2 (half-D passes), normalize, store.

                T_state: the NEXT slot whose W^T transpose groups are
                emitted one per pass boundary, so each group's PSUM/DVE
                copy chase hides behind ready mm2 work instead of
                clogging the PE wait queue.

                fine_tail: narrow final passes for the last drained slot
                so the post-PE evac+DMA+sem chain is as short as possible.
                """
                j, w_sb, rsum, rinv, wts = state
                nblk = EXT[j] // 128
                tq = []
                if T_state is not None:
                    prep_back(T_state)
                    tq = list(range(0, EXT[T_state[0]] // 128, 4))
                passes = ((0, 512), (512, 384), (896, 128)) \
                    if fine_tail else ((0, 512), (512, 512))
                for pi, (dd, dw) in enumerate(passes):
                    if tq:
                        emit_T_group(T_state, tq.pop(0))
                    o_ps = ps_o.tile([128, 512], F32, tag="op")
                    for blk in range(nblk):
                        nc.tensor.matmul(
                            o_ps[:, :dw],
                            wts[blk // 4][:, (blk % 4) * 128:
                                          (blk % 4 + 1) * 128],
                            vnb[blk][:, dd:dd + dw],
                            start=(blk == 0),
                            stop=(blk == nblk - 1),
                        )
                    o_sb = op.tile([128, 512], F16, tag="o")
                    nc.scalar.activation(
                        o_sb[:, :dw], o_ps[:, :dw],
                        mybir.ActivationFunctionType.Copy, scale=rinv[:],
                    )
                    nc.sync.dma_start(
                        o_d[j * 128:(j + 1) * 128, dd:dd + dw], o_sb[:, :dw])
                while tq:
                    emit_T_group(T_state, tq.pop(0))

            # ---- main pipeline -----------------------------------------
            # T (W^T transpose-DMA) at i-2, mm2 at i-3: the T-DMA is
            # emitted before the previous slot's output DMAs so their
            # data waits never hold it up on the SP sequencer.
            states = []
            for i, j in enumerate(ORDER):
                states.append(emit_front(j))
                if i == 2:
                    emit_back_T(states[0])
                if i >= LAG:
                    emit_back_mm2(states[i - LAG], T_state=states[i - 2])
            # drain. ORDER is ascending so the largest slot goes last:
            # its long mm2 hides every earlier slot's output DMA, and
            # only its own (narrowed) final pass sits in the post-PE tail.
            n = len(states)
            for k in range(n - LAG, n):
                t_st = states[k + 1] if k + 1 < n else None
                emit_back_mm2(states[k], T_state=t_st,
                              fine_tail=(k == n - 1))

    nc.finalize()
    return nc


_NC_CACHE = None


def _get_program():
    global _NC_CACHE
    if _NC_CACHE is None:
        _NC_CACHE = _build_program()
    return _NC_CACHE


def stage_inputs(query, value):
    """Build the 8 per-core input maps (partition-major fp16 images)."""
    query = np.asarray(query, dtype=np.float32)
    value = np.asarray(value, dtype=np.float32)

    in_maps = []
    for c in range(NCORES):
        b = c % 4
        tiles = _tiles_for_core(c)
        Q = query[b]
        V = value[b]
        # mask threshold is slot-independent: c - r > t*128 + 256 - EXT
        thr = 128 if c < 4 else 0
        r = np.arange(128)[:, None]
        cc = np.arange(256)[None, :]
        mask_img = np.where(cc - r > thr, MASK_VAL, 0.0).astype(np.float16)

        def chunk_img(ch):
            if ch[0] == "mask":
                return mask_img
            if ch[0] == "qt":
                t = tiles[ch[1]]
                qtile = Q[t * 128:(t + 1) * 128, :]  # [128q, 1024d]
                # per-partition layout: [a(d-chunk) major, q-col minor]
                return qtile.T.reshape(8, 128, 128).transpose(1, 0, 2) \
                    .reshape(128, 8 * 128)
            if ch[0] == "vt":
                c0, w = ch[1], ch[2]
                return V[c0:c0 + w, :].T.reshape(8, 128, w) \
                    .transpose(1, 0, 2).reshape(128, 8 * w)
            r0, n = ch[1], ch[2]
            return V[r0:r0 + n * 128, :].reshape(n, 128, D) \
                .transpose(1, 0, 2).reshape(128, n * D)

        m = {}
        for k, wave in enumerate(WAVES):
            m[f"w{k}"] = np.ascontiguousarray(np.hstack(
                [chunk_img(ch) for ch in wave])).astype(np.float16)
        in_maps.append(m)
    return in_maps


def kernel(query, value):
    nc = _get_program()
    in_maps = stage_inputs(query, value)
    res = run_bass_kernel_spmd(nc, in_maps, core_ids=list(range(NCORES)))

    out = np.empty((B, T, D), dtype=np.float32)
    for c in range(NCORES):
        o = np.asarray(res.results[c]["o"], dtype=np.float32)  # [1024, D]
        b = c % 4
        for j, t in enumerate(_tiles_for_core(c)):
            out[b, t * 128:(t + 1) * 128, :] = o[j * 128:(j + 1) * 128, :]
    return out


# revision 82
# speedup vs baseline: 1.0095x; 1.0095x over previous
"""Causal dense attention (key=value) on 8 TRN2 NeuronCores.

Reference semantics (B=4, T=2048, D=1024, fp32):
    scores  = Q @ V^T                      [B, T, T]
    scores -= 1e9 * (~tril)                causal mask
    W       = softmax(scores, axis=-1)
    out     = W @ V                        [B, T, D]

Sharding: 2 cores per batch; each batch's 16 causal q-tiles (128 rows)
split odd/even across the pair so all 8 cores run one SPMD program with
the padded kv-extent schedule EXT = [256, 512, ..., 2048].

v2 (cost-model driven; 100942 -> 75047 ns):
  - fp16 operands everywhere (mm1, mm2, transposes, output); host converts.
  - inputs staged as partition-major SBUF images packed into 16 wave
    tensors, so every DMA is one fully-contiguous transfer, sequenced so
    each slot's q-tile lands just before the V^T columns it needs.
  - all input DMAs issued up-front on the SP queue; output DMAs queue
    behind them.
  - PE warm-up transposes cover the initial DMA latency so real matmuls
    start at full clock (cost model ramps 0.65 -> 1.2 -> 2.4 GHz over
    3us of continuous PE busy).
  - additive causal mask (-30000, fp16-safe) applied by DVE into PSUM;
    one [128, 256] mask tile serves all slots (threshold is
    slot-independent per core group).
  - per-slot pipeline (LAG=3): slot j's W^T transpose groups are emitted
    at the pass boundaries of slot j-1's mm2, one group per boundary, so
    each group's PSUM->SBUF copy hides behind ready mm2 work instead of
    clogging the PE wait queue.
  - drain ends on the largest slot (its mm2 hides all earlier output
    DMAs) with a narrowed 128-column final pass to minimize the post-PE
    evac -> DMA -> semaphore tail.
"""

import numpy as np

import concourse.bass as bass
import concourse.mybir as mybir
from concourse import bacc, tile
from concourse.bass_utils import run_bass_kernel_spmd
from concourse.masks import make_identity

B, T, D = 4, 2048, 1024
NCORES = 8
NSLOT = 8
EXT = [256 * (j + 1) for j in range(NSLOT)]  # kv extent per slot
# ascending: tiny slots first (fast, low-bandwidth ramp-in), biggest last
ORDER = [0, 1, 2, 3, 4, 5, 6, 7]
LAG = 3
NWARM = 34
MASK_VAL = -30000.0

F32 = mybir.dt.float32
F16 = mybir.dt.float16

# Input DMA waves: each is ONE contiguous partition-major DMA packing
# several logical chunks. Few, larger waves keep the SP sequencer's DMA
# issue phase short so the W^T transpose-DMAs never interleave ahead of
# inputs on the shared HWDGE completion rings.
#   chunk kinds: ("mask",), ("qt", j), ("vt", c0, w), ("vn", r0, nrow)
WAVES = [
    [("qt", 0), ("vt", 0, 128), ("vt", 128, 128)],
    [("mask",), ("qt", 1)],
    [("vt", 256, 128)],
    [("vt", 384, 128)],
    [("qt", 2)],
    [("vt", 512, 256)],
    [("qt", 3)],
    [("vt", 768, 256)],
    [("qt", 4)],
    [("vt", 1024, 256), ("vn", 0, 2)],
    [("qt", 5), ("vt", 1280, 256), ("vn", 256, 2)],
    [("vn", 512, 4)],
    [("qt", 6), ("vt", 1536, 256)],
    [("qt", 7), ("vt", 1792, 256)],
    [("vn", 1024, 4)],
    [("vn", 1536, 4)],
]


def _chunk_cols(ch):
    if ch[0] == "mask":
        return 256
    if ch[0] == "qt":
        return 8 * 128
    if ch[0] == "vt":
        return 8 * ch[2]
    return ch[2] * D  # vn


def _wave_cols(wave):
    return sum(_chunk_cols(ch) for ch in wave)


def _tiles_for_core(c):
    """q-tile index (within the batch) for each slot, for core c."""
    if c < 4:
        return [2 * j + 1 for j in range(NSLOT)]  # extents exactly EXT
    return [2 * j for j in range(NSLOT)]  # extents EXT - 128 (padded)


VT_SPANS = [(ch[1], ch[2]) for wave in WAVES for ch in wave if ch[0] == "vt"]


def _segments(c0, w):
    """Split window [c0, c0+w) at vt-chunk boundaries -> (span, off, g0, gw)."""
    segs = []
    for si, (s0, sw) in enumerate(VT_SPANS):
        lo = max(c0, s0)
        hi = min(c0 + w, s0 + sw)
        if lo < hi:
            segs.append((si, lo - s0, lo - c0, hi - lo))
    return segs


def _build_program():
    nc = bacc.Bacc("TRN2", target_bir_lowering=False)

    wave_d = [nc.dram_tensor(f"w{k}", [128, _wave_cols(wave)], F16,
                             kind="ExternalInput")
              for k, wave in enumerate(WAVES)]
    o_d = nc.dram_tensor("o", [NSLOT * 128, D], F16, kind="ExternalOutput")

    with tile.TileContext(nc) as tc:
        with (
            tc.tile_pool(name="const", bufs=1) as constp,
            tc.tile_pool(name="vt", bufs=1) as vtp,
            tc.tile_pool(name="w", bufs=4) as wp,
            tc.tile_pool(name="wt", bufs=8) as wtp,
            tc.tile_pool(name="osb", bufs=8) as op,
            tc.tile_pool(name="stats", bufs=24) as statp,
            tc.tile_pool(name="ps_s", bufs=4, space="PSUM") as ps_s,
            tc.tile_pool(name="ps_t", bufs=2, space="PSUM") as ps_t,
            tc.tile_pool(name="ps_o", bufs=2, space="PSUM") as ps_o,
        ):
            # warm-source: zeros are fine, warm-up results are never read
            warmsrc = constp.tile([128, 128], F16, tag="warmsrc")
            nc.gpsimd.memset(warmsrc[:], 0.0)

            ident = constp.tile([128, 128], F16, tag="ident")
            make_identity(nc, ident[:])

            # ACT exp-table warm-up during initial DMAs
            warm_a = statp.tile([128, 1], F32, tag="warma")
            nc.gpsimd.memset(warm_a[:], 0.0)
            nc.scalar.activation(warm_a[:], warm_a[:],
                                 mybir.ActivationFunctionType.Exp)

            # ---- all input DMAs, in slot-consumption order -------------
            qts = {}   # j -> AP [128, 8*128]   (d8-major q-tile image)
            vtc = []   # vt span idx -> (tile, off, w)
            vnb = {}   # 128-row block index -> AP [128, D]
            maskc = None

            for k, wave in enumerate(WAVES):
                t_ = vtp.tile([128, _wave_cols(wave)], F16, tag=f"wv{k}")
                nc.sync.dma_start(t_[:], wave_d[k][:])
                off = 0
                for ch in wave:
                    cols = _chunk_cols(ch)
                    if ch[0] == "mask":
                        maskc = t_[:, off:off + 256]
                    elif ch[0] == "qt":
                        qts[ch[1]] = (t_, off)
                    elif ch[0] == "vt":
                        vtc.append((t_, off, ch[2]))
                    else:  # vn
                        for i in range(ch[2]):
                            vnb[ch[1] // 128 + i] = t_[:, off + i * D:
                                                       off + (i + 1) * D]
                    off += cols

            # ---- PE warm-up: junk transposes cover the DMA head --------
            for _ in range(NWARM):
                t_ps = ps_t.tile([128, 512], F16, tag="tp",
                                 padded_shape=[128, 1024])
                nc.tensor.transpose(t_ps[:, 0:128], warmsrc[:], warmsrc[:])

            def emit_junk(n):
                """junk transposes: keep the PE queue fed across known
                data-bound head gaps (53ns each, results never read)."""
                for _ in range(n):
                    t_ps = ps_t.tile([128, 512], F16, tag="tp",
                                     padded_shape=[128, 1024])
                    nc.tensor.transpose(t_ps[:, 0:128], warmsrc[:],
                                        warmsrc[:])

            def emit_front(j, junk_mid=0):
                """mm1 + softmax stats + exp for slot j."""
                E = EXT[j]
                windows = [(c0, min(512, E - c0)) for c0 in range(0, E, 512)]
                nmax = None
                s_list = []
                for c0, w in windows:
                    s_ = ps_s.tile([128, 512], F32, tag="sw")
                    last = (c0 + w == E)
                    segs = _segments(c0, w)
                    qt_t, qt_off = qts[j]
                    for si, (sp, off, g0, gw) in enumerate(segs):
                        vt_t, vt_off, vw = vtc[sp]
                        for d8 in range(8):
                            nc.tensor.matmul(
                                s_[:, g0:g0 + gw],
                                qt_t[:, qt_off + d8 * 128:
                                     qt_off + (d8 + 1) * 128],
                                vt_t[:, vt_off + d8 * vw + off:
                                     vt_off + d8 * vw + off + gw],
                                start=(d8 == 0 and g0 == 0),
                                stop=(si == len(segs) - 1 and d8 == 7),
                            )
                    # additive causal mask on the last 256 columns (DVE)
                    if last:
                        nc.vector.tensor_add(s_[:, w - 256:w],
                                             s_[:, w - 256:w], maskc[:])
                    nm = statp.tile([128, 1], F32, tag="nm")
                    nc.vector.reduce_max(nm[:], s_[:, :w],
                                         axis=mybir.AxisListType.X, negate=True)
                    if nmax is None:
                        nmax = nm
                    else:
                        nm2 = statp.tile([128, 1], F32, tag="nmc")
                        nc.vector.tensor_tensor(nm2[:], nmax[:], nm[:],
                                                op=mybir.AluOpType.min)
                        nmax = nm2
                    s_list.append((c0, w, s_))

                w_sb = wp.tile([128, E], F16, tag="w")
                rsum = None
                for c0, w, s_ in s_list:
                    rs = statp.tile([128, 1], F32, tag="rs")
                    nc.scalar.activation(
                        w_sb[:, c0:c0 + w], s_[:, :w],
                        mybir.ActivationFunctionType.Exp,
                        bias=nmax[:], accum_out=rs[:],
                    )
                    if rsum is None:
                        rsum = rs
                    else:
                        rs2 = statp.tile([128, 1], F32, tag="rsc")
                        nc.vector.tensor_add(rs2[:], rsum[:], rs[:])
                        rsum = rs2
                return [j, w_sb, rsum, None, None]

            def prep_back(state):
                """1/rowsum (deferred off the hot DVE stats path) and the
                W^T group list for a finished slot."""
                j, w_sb, rsum, _, _ = state
                rinv = statp.tile([128, 1], F32, tag="rinv")
                nc.vector.reciprocal(rinv[:], rsum[:])
                state[3] = rinv
                state[4] = []

            def emit_T_group(state, g0):
                """PE-transpose one group of 4 W blocks + DVE copy to SBUF."""
                j, w_sb, rsum, rinv, wts = state
                nblk = EXT[j] // 128
                gn = min(4, nblk - g0)
                t_ps = ps_t.tile([128, 512], F16, tag="tp",
                                 padded_shape=[128, 1024])
                for bi in range(gn):
                    blk = g0 + bi
                    nc.tensor.transpose(
                        t_ps[:, bi * 128:(bi + 1) * 128],
                        w_sb[:, blk * 128:(blk + 1) * 128],
                        ident[:],
                    )
                wt = wtp.tile([128, 512], F16, tag="wt")
                nc.vector.tensor_copy(wt[:, :gn * 128], t_ps[:, :gn * 128])
                wts.append(wt)

            def emit_back_T(state):
                prep_back(state)
                for g0 in range(0, EXT[state[0]] // 128, 4):
                    emit_T_group(state, g0)

            def emit_back_mm2(state, T_state=None, fine_tail=False):
                """mm2 (half-D passes), normalize, store.

                T_state: the NEXT slot whose W^T transpose groups are
                emitted one per pass boundary, so each group's PSUM/DVE
                copy chase hides behind ready mm2 work instead of
                clogging the PE wait queue.

                fine_tail: narrow final passes for the last drained slot
                so the post-PE evac+DMA+sem chain is as short as possible.
                """
                j, w_sb, rsum, rinv, wts = state
                nblk = EXT[j] // 128
                tq = []
                if T_state is not None:
                    prep_back(T_state)
                    tq = list(range(0, EXT[T_state[0]] // 128, 4))
                passes = ((0, 512), (512, 384), (896, 128)) \
                    if fine_tail else ((0, 512), (512, 512))
                for pi, (dd, dw) in enumerate(passes):
                    if tq:
                        emit_T_group(T_state, tq.pop(0))
                    o_ps = ps_o.tile([128, 512], F32, tag="op")
                    for blk in range(nblk):
                        nc.tensor.matmul(
                            o_ps[:, :dw],
                            wts[blk // 4][:, (blk % 4) * 128:
                                          (blk % 4 + 1) * 128],
                            vnb[blk][:, dd:dd + dw],
                            start=(blk == 0),
                            stop=(blk == nblk - 1),
                        )
                    o_sb = op.tile([128, 512], F16, tag="o")
                    # evac on DVE: the in-order ACT queue would park this
                    # behind pending exp chains, delaying ps_o recycling
                    nc.vector.tensor_scalar_mul(
                        o_sb[:, :dw], o_ps[:, :dw], rinv[:])
                    nc.sync.dma_start(
                        o_d[j * 128:(j + 1) * 128, dd:dd + dw], o_sb[:, :dw])
                while tq:
                    emit_T_group(T_state, tq.pop(0))

            # ---- main pipeline -----------------------------------------
            # T (W^T transpose-DMA) at i-2, mm2 at i-3: the T-DMA is
            # emitted before the previous slot's output DMAs so their
            # data waits never hold it up on the SP sequencer.
            states = []
            for i, j in enumerate(ORDER):
                states.append(emit_front(j))
                if i == 2:
                    emit_back_T(states[0])
                if i >= LAG:
                    emit_back_mm2(states[i - LAG], T_state=states[i - 2])
            # drain. ORDER is ascending so the largest slot goes last:
            # its long mm2 hides every earlier slot's output DMA, and
            # only its own (narrowed) final pass sits in the post-PE tail.
            n = len(states)
            for k in range(n - LAG, n):
                t_st = states[k + 1] if k + 1 < n else None
                emit_back_mm2(states[k], T_state=t_st,
                              fine_tail=(k == n - 1))

    nc.finalize()
    return nc


_NC_CACHE = None


def _get_program():
    global _NC_CACHE
    if _NC_CACHE is None:
        _NC_CACHE = _build_program()
    return _NC_CACHE


def stage_inputs(query, value):
    """Build the 8 per-core input maps (partition-major fp16 images)."""
    query = np.asarray(query, dtype=np.float32)
    value = np.asarray(value, dtype=np.float32)

    in_maps = []
    for c in range(NCORES):
        b = c % 4
        tiles = _tiles_for_core(c)
        Q = query[b]
        V = value[b]
        # mask threshold is slot-independent: c - r > t*128 + 256 - EXT
        thr = 128 if c < 4 else 0
        r = np.arange(128)[:, None]
        cc = np.arange(256)[None, :]
        mask_img = np.where(cc - r > thr, MASK_VAL, 0.0).astype(np.float16)

        def chunk_img(ch):
            if ch[0] == "mask":
                return mask_img
            if ch[0] == "qt":
                t = tiles[ch[1]]
                qtile = Q[t * 128:(t + 1) * 128, :]  # [128q, 1024d]
                # per-partition layout: [a(d-chunk) major, q-col minor]
                return qtile.T.reshape(8, 128, 128).transpose(1, 0, 2) \
                    .reshape(128, 8 * 128)
            if ch[0] == "vt":
                c0, w = ch[1], ch[2]
                return V[c0:c0 + w, :].T.reshape(8, 128, w) \
                    .transpose(1, 0, 2).reshape(128, 8 * w)
            r0, n = ch[1], ch[2]
            return V[r0:r0 + n * 128, :].reshape(n, 128, D) \
                .transpose(1, 0, 2).reshape(128, n * D)

        m = {}
        for k, wave in enumerate(WAVES):
            m[f"w{k}"] = np.ascontiguousarray(np.hstack(
                [chunk_img(ch) for ch in wave])).astype(np.float16)
        in_maps.append(m)
    return in_maps


def kernel(query, value):
    nc = _get_program()
    in_maps = stage_inputs(query, value)
    res = run_bass_kernel_spmd(nc, in_maps, core_ids=list(range(NCORES)))

    out = np.empty((B, T, D), dtype=np.float32)
    for c in range(NCORES):
        o = np.asarray(res.results[c]["o"], dtype=np.float32)  # [1024, D]
        b = c % 4
        for j, t in enumerate(_tiles_for_core(c)):
            out[b, t * 128:(t + 1) * 128, :] = o[j * 128:(j + 1) * 128, :]
    return out
